# revision 4
# baseline (speedup 1.0000x reference)
"""Trainium2 Bass kernel for nn_BasicBlock (binarized 3x3 conv + BN + ReLU).

Reference computation (NHWC, f32):
    a   = ste_sign(x + bias1)            # +-1, sign(0)=+1
    qk  = ste_sign(kernel)               # +-1
    y   = conv2d(a, qk, SAME, stride 1)  # (32,56,56,256)
    y   = (y - mean) * rsqrt(var+eps) + beta
    out = relu(y + bias2)

Strategy:
  - Data-parallel over batch: 8 cores x 4 images, no collectives.
  - All matmul operands are exactly +-1 so bf16 matmuls with fp32 PSUM
    accumulation are bit-exact (sums are integers |y| <= 2304 < 2^24).
  - Per core: DMA NHWC rows -> PE transpose (f32) to channel-major ->
    ScalarE Sign(x + bias1) into a zero-padded (58-wide) image buffer in
    bf16 -> conv as 18 accumulated matmuls per 2-row output tile
    (9 taps x 2 Cin tiles; stationary = activations slice, moving =
    binarized weights, psum out = [pixel, Cout]) -> VectorE fused
    (y + t/s) * s with relu -> strided NHWC stores.
  - Host precomputes constants only: sign(kernel) in bf16, folded BN
    scale/shift, and a 1-ulp nudge of x where fl(x+bias1)==0 so the
    device Sign (sign(0)=0) matches ste_sign (sign(0)=+1).
"""

import numpy as np
import ml_dtypes

import concourse.bass as bass
import concourse.mybir as mybir
import concourse.tile as tile
from concourse import bacc
from concourse.bass_utils import run_bass_kernel_spmd
from concourse.masks import make_identity

# Problem shape (hardcoded per contract).
B, H, W, CIN, COUT = 32, 56, 56, 256, 256
N_CORES = 8
IMG = B // N_CORES          # images per core
EPS = 1e-3

P = 128
WPAD = 58                   # padded row width (56 + 2)
ROWS = 59                   # padded rows incl. 1 top pad, 56 real, 1 bottom pad + slack
AFREE = ROWS * WPAD         # 3422 flat elems per (ci_tile); max mm read = 3377
RP = H // 2                 # 28 row-pairs per image
G4 = H // 4                 # 14 four-row load groups per image

F32 = mybir.dt.float32
BF16 = mybir.dt.bfloat16


def _build_program():
    nc = bacc.Bacc(
        "TRN2",
        target_bir_lowering=False,
        debug=False,
        enable_asserts=False,
        num_devices=N_CORES,
    )

    x_ap = nc.dram_tensor("x", (IMG, H, W, CIN), F32, kind="ExternalInput").ap()
    w_ap = nc.dram_tensor("wq", (2, P, 9, COUT), BF16, kind="ExternalInput").ap()
    b1_ap = nc.dram_tensor("b1", (2, P), F32, kind="ExternalInput").ap()
    s2_ap = nc.dram_tensor("s2", (COUT,), F32, kind="ExternalInput").ap()
    t2_ap = nc.dram_tensor("t2", (COUT,), F32, kind="ExternalInput").ap()
    out_ap = nc.dram_tensor("out", (IMG, H, W, COUT), F32, kind="ExternalOutput").ap()

    x_flat = x_ap.rearrange("b h w c -> b (h w) c")

    with tile.TileContext(nc) as tc:
        with (
            tc.tile_pool(name="const", bufs=1) as const_pool,
            tc.tile_pool(name="xin", bufs=4) as x_pool,
            tc.tile_pool(name="uout", bufs=4) as u_pool,
            tc.tile_pool(name="pst", bufs=3, space="PSUM") as pst_pool,
            tc.tile_pool(name="pso", bufs=4, space="PSUM") as pso_pool,
        ):
            w_sb = const_pool.tile([P, 2, 9, COUT], BF16)
            nc.sync.dma_start(w_sb[:], w_ap.rearrange("t p a c -> p t a c"))
            b1_sb = const_pool.tile([P, 2], F32)
            nc.sync.dma_start(b1_sb[:], b1_ap.rearrange("t p -> p t"))
            s2_sb = const_pool.tile([P, COUT], F32)
            nc.sync.dma_start(s2_sb[:], s2_ap[None, :].to_broadcast((P, COUT)))
            t2_sb = const_pool.tile([P, COUT], F32)
            nc.sync.dma_start(t2_sb[:], t2_ap[None, :].to_broadcast((P, COUT)))
            ident = const_pool.tile([P, P], F32)
            make_identity(nc, ident[:])

            # Double-buffered padded activation buffers, one per image slot.
            acts = [
                const_pool.tile([P, 2, AFREE], BF16, name=f"act{i}") for i in range(2)
            ]
            nc.gpsimd.memset(acts[0][:], 0.0)
            nc.gpsimd.memset(acts[1][:], 0.0)

            for b in range(IMG):
                act = acts[b % 2]
                act_rows = [
                    act[:, ci].rearrange("p (r c) -> p r c", c=WPAD) for ci in range(2)
                ]

                # ---- load + transpose + binarize ----
                for g in range(G4):
                    xt = x_pool.tile([112, 2, CIN], F32)
                    nc.sync.dma_start(
                        xt[:],
                        x_flat[b, 224 * g : 224 * (g + 1), :].rearrange(
                            "(j p) c -> p j c", p=112
                        ),
                    )
                    for j in range(2):
                        k = 2 * g + j  # row-pair index
                        for ci in range(2):
                            pt = pst_pool.tile([P, 112], F32)
                            nc.tensor.transpose(
                                pt[:],
                                xt[:, j, ci * P : (ci + 1) * P],
                                ident[:112, :112],
                            )
                            # a = sign(xT + bias1) in bf16, into padded rows
                            # (2k+1, 2k+2), cols 2..57.
                            nc.scalar.activation(
                                act_rows[ci][:, 2 * k + 1 : 2 * k + 3, 2:58],
                                pt.rearrange("p (r c) -> p r c", c=56),
                                mybir.ActivationFunctionType.Sign,
                                bias=b1_sb[:, ci : ci + 1],
                                scale=1.0,
                            )

                # ---- conv + BN + relu + store, one 2-row tile at a time ----
                for k in range(RP):
                    po = pso_pool.tile([P, COUT], F32)
                    acc = 0
                    for dh in range(3):
                        for dw in range(3):
                            base = (2 * k + dh) * WPAD + dw
                            for ci in range(2):
                                nc.tensor.matmul(
                                    po[:],
                                    act[:, ci, base : base + P],
                                    w_sb[:, ci, 3 * dh + dw, :],
                                    start=(acc == 0),
                                    stop=(acc == 17),
                                )
                                acc += 1
                    u = u_pool.tile([P, COUT], F32)
                    # u = (y + t/s); out rows = relu(u) * s  == relu(y*s + t)
                    nc.vector.tensor_tensor(u[:], po[:], t2_sb[:], mybir.AluOpType.add)
                    nc.vector.scalar_tensor_tensor(
                        u[:], u[:], 0.0, s2_sb[:],
                        op0=mybir.AluOpType.max, op1=mybir.AluOpType.mult,
                    )
                    # psum partitions: 0 pad | 1..56 row 2k | 57,58 pad |
                    # 59..114 row 2k+1 | 115..127 garbage
                    nc.sync.dma_start(out_ap[b, 2 * k], u[1:57, :])
                    nc.sync.dma_start(out_ap[b, 2 * k + 1], u[59:115, :])

    nc.compile()
    return nc


_NC_CACHE = None


def _get_nc():
    global _NC_CACHE
    if _NC_CACHE is None:
        _NC_CACHE = _build_program()
    return _NC_CACHE


def _prep_inputs(x, bias1, kernel, bn_beta, bn_mean, bn_var, bias2):
    x = np.asarray(x, dtype=np.float32)
    bias1 = np.asarray(bias1, dtype=np.float32)
    kernel = np.asarray(kernel, dtype=np.float32)
    bn_beta = np.asarray(bn_beta, dtype=np.float32)
    bn_mean = np.asarray(bn_mean, dtype=np.float32)
    bn_var = np.asarray(bn_var, dtype=np.float32)
    bias2 = np.asarray(bias2, dtype=np.float32).reshape(-1)

    # Device computes sign(fl(x + b)) with sign(0)=0; the reference wants
    # sign(0)=+1. Nudge x by 1 ulp wherever fl(x+b) == 0 exactly (x is only
    # consumed through this sign).
    z = x + bias1
    if np.any(z == 0.0):
        x = np.where(z == 0.0, np.nextafter(x, np.float32(np.inf)), x)

    # Weights: ste_sign with sign(0)=+1, exact in bf16.
    wq = np.where(kernel >= 0, np.float32(1.0), np.float32(-1.0))
    # [kh,kw,ci,co] -> [ci_tile, ci_in, tap, co]
    wq = np.ascontiguousarray(
        wq.transpose(2, 0, 1, 3).reshape(2, P, 9, COUT)
    ).astype(ml_dtypes.bfloat16)

    s = (1.0 / np.sqrt(bn_var + np.float32(EPS))).astype(np.float32)
    t = (bn_beta - bn_mean * s + bias2).astype(np.float32)
    t2 = (t / s).astype(np.float32)
    b1 = np.ascontiguousarray(bias1.reshape(2, P)).astype(np.float32)

    in_maps = []
    for c in range(N_CORES):
        in_maps.append(
            {
                "x": np.ascontiguousarray(x[c * IMG : (c + 1) * IMG]),
                "wq": wq,
                "b1": b1,
                "s2": s,
                "t2": t2,
            }
        )
    return in_maps


def _ensure_ntff_hook():
    """This container ships the NTFF profiling machinery but not the
    ``antenv.axon_hooks`` shim module bass_utils imports it through;
    synthesize it so trace=True can capture HW exec times."""
    import sys
    import types

    if "antenv.axon_hooks" in sys.modules:
        return
    import antenv
    from trn_agent_boot.trn_boot import _ntff_profile_via_ctypes

    hook = _ntff_profile_via_ctypes("/opt/axon/libaxon_pjrt.so")
    mod = types.ModuleType("antenv.axon_hooks")
    mod.get_axon_ntff_profile_hook = lambda: hook
    mod.set_axon_ntff_profile_hook = lambda h: None
    sys.modules["antenv.axon_hooks"] = mod
    antenv.axon_hooks = mod


def run(inputs: dict, trace: bool = False):
    """Run the SPMD kernel. Returns (out, exec_time_ns or None)."""
    nc = _get_nc()
    in_maps = _prep_inputs(**inputs)
    if trace:
        try:
            _ensure_ntff_hook()
        except Exception as e:  # degrade to untraced run
            print(f"ntff hook unavailable: {e}")
    res = run_bass_kernel_spmd(
        nc, in_maps, core_ids=list(range(N_CORES)), trace=trace
    )
    out = np.concatenate([r["out"] for r in res.results], axis=0)
    return out, res.exec_time_ns


def kernel(**inputs) -> np.ndarray:
    out, _ = run(inputs, trace=False)
    return out


# revision 7
# speedup vs baseline: 1.0730x; 1.0730x over previous
"""Trainium2 Bass kernel for nn_BasicBlock (binarized 3x3 conv + BN + ReLU).

Reference computation (NHWC, f32):
    a   = ste_sign(x + bias1)            # +-1, sign(0)=+1
    qk  = ste_sign(kernel)               # +-1
    y   = conv2d(a, qk, SAME, stride 1)  # (32,56,56,256)
    y   = (y - mean) * rsqrt(var+eps) + beta
    out = relu(y + bias2)

Strategy (v2, fp8 DoubleRow):
  - Data-parallel over batch: 8 cores x 4 images, no collectives.
  - Operands are exactly +-1, exact in fp8e4; fp32 PSUM accumulation keeps
    integer conv sums (|y| <= 2304) bit-exact. DoubleRow packs both
    128-channel Cin halves into one matmul at 2 MACs/cell/cycle.
  - Per core pipeline, per image:
      load 8-row groups [112 part, 4 px, 256ch] (4KB/partition descriptors)
      -> PE transpose f32 -> ScalarE Sign(x+bias1) -> fp8 +-1 into a
      zero-padded 58-wide channel-major image buffer
      -> conv: weights-stationary fp8 DoubleRow matmuls, psum [co,464px]
         (8 output rows per group, 9 taps accumulated, 2 Cout tiles)
      -> VectorE BN affine (y*s + t, per-partition scale/shift)
      -> PE transpose back to [px, co] per 2-row tile
      -> VectorE fused relu + PSUM evacuation into a per-image staging
         buffer -> 2 large NHWC stores per image via GPSIMD (SWDGE).
  - Host precomputes constants only: sign(kernel) in fp8 DoubleRow layout,
    folded BN scale/shift, and a 1-ulp nudge of x where fl(x+bias1)==0 so
    device Sign (sign(0)=0) matches ste_sign (sign(0)=+1).
"""

import numpy as np
import ml_dtypes

import concourse.bass as bass
import concourse.mybir as mybir
import concourse.tile as tile
from concourse import bacc
from concourse.bass_utils import run_bass_kernel_spmd
from concourse.masks import make_identity

# Problem shape (hardcoded per contract).
B, H, W, CIN, COUT = 32, 56, 56, 256, 256
N_CORES = 8
IMG = B // N_CORES          # images per core
EPS = 1e-3

P = 128
WPAD = 58                   # padded row width (56 + 2)
ROWS = 59                   # 1 top pad + 56 real + 1 bottom pad + slack
AFREE = 3424                # ROWS*WPAD=3422 padded to %16 for DoubleRow APs
RP = H // 2                 # 28 row-pairs per image
G8 = H // 8                 # 7 eight-row groups per image
NPX = 8 * WPAD              # 464 psum pixels per conv group

F32 = mybir.dt.float32
FP8 = mybir.dt.float8e4

AluOp = mybir.AluOpType


def _build_program():
    nc = bacc.Bacc(
        "TRN2",
        target_bir_lowering=False,
        debug=False,
        enable_asserts=False,
        num_devices=N_CORES,
    )

    x_ap = nc.dram_tensor("x", (IMG, H, W, CIN), F32, kind="ExternalInput").ap()
    w_ap = nc.dram_tensor("wq", (P, 9, 2, 2, P), FP8, kind="ExternalInput").ap()
    b1_ap = nc.dram_tensor("b1", (2, P), F32, kind="ExternalInput").ap()
    s_ap = nc.dram_tensor("s", (2, P), F32, kind="ExternalInput").ap()
    t_ap = nc.dram_tensor("t", (2, P), F32, kind="ExternalInput").ap()
    out_ap = nc.dram_tensor("out", (IMG, H, W, COUT), F32, kind="ExternalOutput").ap()

    x_flat = x_ap.rearrange("b h w c -> b (h w) c")

    with tile.TileContext(nc) as tc:
        with (
            tc.tile_pool(name="const", bufs=1) as const_pool,
            tc.tile_pool(name="xin", bufs=4) as x_pool,
            tc.tile_pool(name="ybn", bufs=16) as y_pool,
            tc.tile_pool(name="pst", bufs=2, space="PSUM") as pst_pool,
            tc.tile_pool(name="pso", bufs=4, space="PSUM") as pso_pool,
            tc.tile_pool(name="psu", bufs=2, space="PSUM") as psu_pool,
        ):
            w_sb = const_pool.tile([P, 9, 2, 2, P], FP8)
            nc.sync.dma_start(w_sb[:], w_ap)
            b1_sb = const_pool.tile([P, 2], F32)
            nc.sync.dma_start(b1_sb[:], b1_ap.rearrange("t p -> p t"))
            s_sb = const_pool.tile([P, 2], F32)
            nc.sync.dma_start(s_sb[:], s_ap.rearrange("t p -> p t"))
            t_sb = const_pool.tile([P, 2], F32)
            nc.sync.dma_start(t_sb[:], t_ap.rearrange("t p -> p t"))
            ident = const_pool.tile([P, P], F32)
            make_identity(nc, ident[:])

            # Double-buffered fp8 padded activation buffers + f32 output stage.
            acts = [
                const_pool.tile([P, 2, AFREE], FP8, name=f"act{i}") for i in range(2)
            ]
            ubig = [
                const_pool.tile([P, RP, COUT], F32, name=f"ubig{i}") for i in range(2)
            ]
            nc.gpsimd.memset(acts[0][:], 0.0)
            nc.gpsimd.memset(acts[1][:], 0.0)

            for b in range(IMG):
                slot = b % 2
                act = acts[slot]
                ub = ubig[slot]

                # ---- load + transpose + binarize (8-row groups) ----
                for g in range(G8):
                    xt = x_pool.tile([112, 4, CIN], F32)
                    nc.sync.dma_start(
                        xt[:],
                        x_flat[b, 448 * g : 448 * (g + 1), :].rearrange(
                            "(p j) c -> p j c", p=112
                        ),
                    )
                    for j in range(4):
                        for ci in range(2):
                            pt = pst_pool.tile([P, 112], F32)
                            nc.tensor.transpose(
                                pt[:],
                                xt[:, j, ci * P : (ci + 1) * P],
                                ident[:112, :112],
                            )
                            # px = 4p + j -> row r = px//56 (8 rows), col 4q+j.
                            # dest: padded rows 8g+1..8g+8, cols 2+j step 4.
                            base = (8 * g + 1) * WPAD + 2 + j
                            dest = (
                                act[:, ci, base : base + 8 * WPAD]
                                .rearrange("p (r w) -> p r w", w=WPAD)[:, :, 0:56]
                                .rearrange("p r (q x) -> p r q x", x=4)[:, :, :, 0]
                            )
                            nc.scalar.activation(
                                dest,
                                pt.rearrange("p (r q) -> p r q", q=14),
                                mybir.ActivationFunctionType.Sign,
                                bias=b1_sb[:, ci : ci + 1],
                                scale=1.0,
                            )

                # ---- conv (fp8 DoubleRow, weights stationary) + BN affine ----
                y_tiles = {}
                for co in range(2):
                    for block in ((0, 1, 2, 3), (4, 5, 6)):
                        pm = {
                            m: pso_pool.tile([P, NPX], F32, name="pm", tag="pm")
                            for m in block
                        }
                        for tap in range(9):
                            dh, dw = tap // 3, tap % 3
                            for m in block:
                                rbase = (8 * m + dh) * WPAD + dw
                                nc.tensor.matmul(
                                    pm[m][:],
                                    w_sb[:, tap, co],
                                    act[:, :, rbase : rbase + NPX],
                                    start=(tap == 0),
                                    stop=(tap == 8),
                                    perf_mode=mybir.MatmulPerfMode.DoubleRow,
                                )
                        for m in block:
                            y = y_pool.tile([P, NPX], F32, name="y", tag="y")
                            # y = conv * scale + shift   (per-partition co consts)
                            nc.vector.tensor_scalar(
                                y[:], pm[m][:],
                                s_sb[:, co : co + 1], t_sb[:, co : co + 1],
                                op0=AluOp.mult, op1=AluOp.add,
                            )
                            y_tiles[(co, m)] = y

                # ---- transpose back to [px, co], fused relu, stage, store ----
                for m in range(G8):
                    for r in range(4):
                        k = 4 * m + r
                        pu = psu_pool.tile([116, COUT], F32)
                        for co in range(2):
                            nc.tensor.matmul(
                                pu[:, co * P : (co + 1) * P],
                                y_tiles[(co, m)][:, 116 * r : 116 * r + 116],
                                ident[:, :P],
                                is_transpose=True,
                                start=(co == 0),
                                stop=(co == 1),
                            )
                        # partitions: 0 pad | 1..56 row 2k | 57,58 pad |
                        # 59..114 row 2k+1 | 115 pad
                        nc.vector.tensor_scalar(
                            ub[:116, k, :], pu[:], 0.0, None, op0=AluOp.max
                        )

                ev = out_ap[b].rearrange("(k two) w c -> w two k c", two=2)
                nc.gpsimd.dma_start(ev[:, 0], ub[1:57])
                nc.gpsimd.dma_start(ev[:, 1], ub[59:115])

    nc.compile()
    return nc


_NC_CACHE = None


def _get_nc():
    global _NC_CACHE
    if _NC_CACHE is None:
        _NC_CACHE = _build_program()
    return _NC_CACHE


def _prep_inputs(x, bias1, kernel, bn_beta, bn_mean, bn_var, bias2):
    x = np.asarray(x, dtype=np.float32)
    bias1 = np.asarray(bias1, dtype=np.float32)
    kernel = np.asarray(kernel, dtype=np.float32)
    bn_beta = np.asarray(bn_beta, dtype=np.float32)
    bn_mean = np.asarray(bn_mean, dtype=np.float32)
    bn_var = np.asarray(bn_var, dtype=np.float32)
    bias2 = np.asarray(bias2, dtype=np.float32).reshape(-1)

    # Device computes sign(fl(x + b)) with sign(0)=0; the reference wants
    # sign(0)=+1. Nudge x by 1 ulp wherever fl(x+b) == 0 exactly (x is only
    # consumed through this sign).
    z = x + bias1
    if np.any(z == 0.0):
        x = np.where(z == 0.0, np.nextafter(x, np.float32(np.inf)), x)

    # Weights: ste_sign with sign(0)=+1, exact in fp8e4.
    # [kh,kw,ci,co] -> [ki, tap, co_t, o(ci half), co] (DoubleRow pairing
    # puts ci = o*128 + ki, matching the act buffer's [ki, ci_t, px] layout).
    wq = np.where(kernel >= 0, np.float32(1.0), np.float32(-1.0))
    wq = wq.reshape(9, 2, P, 2, P).transpose(2, 0, 3, 1, 4)
    wq = np.ascontiguousarray(wq).astype(ml_dtypes.float8_e4m3)

    s = (1.0 / np.sqrt(bn_var + np.float32(EPS))).astype(np.float32)
    t = (bn_beta - bn_mean * s + bias2).astype(np.float32)
    b1 = np.ascontiguousarray(bias1.reshape(2, P)).astype(np.float32)

    in_maps = []
    for c in range(N_CORES):
        in_maps.append(
            {
                "x": np.ascontiguousarray(x[c * IMG : (c + 1) * IMG]),
                "wq": wq,
                "b1": b1,
                "s": np.ascontiguousarray(s.reshape(2, P)),
                "t": np.ascontiguousarray(t.reshape(2, P)),
            }
        )
    return in_maps


def _ensure_ntff_hook():
    """This container ships the NTFF profiling machinery but not the
    ``antenv.axon_hooks`` shim module bass_utils imports it through;
    synthesize it so trace=True can capture HW exec times."""
    import sys
    import types

    if "antenv.axon_hooks" in sys.modules:
        return
    import antenv
    from trn_agent_boot.trn_boot import _ntff_profile_via_ctypes

    hook = _ntff_profile_via_ctypes("/opt/axon/libaxon_pjrt.so")
    mod = types.ModuleType("antenv.axon_hooks")
    mod.get_axon_ntff_profile_hook = lambda: hook
    mod.set_axon_ntff_profile_hook = lambda h: None
    sys.modules["antenv.axon_hooks"] = mod
    antenv.axon_hooks = mod


def run(inputs: dict, trace: bool = False):
    """Run the SPMD kernel. Returns (out, exec_time_ns or None)."""
    nc = _get_nc()
    in_maps = _prep_inputs(**inputs)
    if trace:
        try:
            _ensure_ntff_hook()
        except Exception as e:  # degrade to untraced run
            print(f"ntff hook unavailable: {e}")
    res = run_bass_kernel_spmd(
        nc, in_maps, core_ids=list(range(N_CORES)), trace=trace
    )
    out = np.concatenate([r["out"] for r in res.results], axis=0)
    return out, res.exec_time_ns


def kernel(**inputs) -> np.ndarray:
    out, _ = run(inputs, trace=False)
    return out
